# revision 19
# baseline (speedup 1.0000x reference)
"""Trainium2 Bass kernel for nn_NetworkActivity_layer (masked linear):

    out = x @ (weight * mask.T).T + bias      x:(4096,15000) w:(500,15000)
                                              mask:(15000,500) bias:(500,)

Strategy: shard the contraction (gene) dim K=15000 across 8 NeuronCores
(1875 genes/core). Each core computes a partial (4096,500) output; the
host sums the 8 partials (the K-shard "unshard" step). The bias rides in
an extra padded gene (x column of ones, masked-weight row = bias on core
0), so the device kernel computes the complete affine map.

The masked weights are premultiplied on the host (weight * mask.T) so the
device does nothing but matmuls. The kernel is PE-streaming-bound, so the
contraction is split by precision to cut PE cycles while holding the
error budget (rel err gate 2e-2):
  - 11 k-tiles (1407 genes + bias) in bf16: 1 matmul each, N=500.
  - 4 k-tiles (468 genes + 44 zero pads) in fp8 e4m3 with
    perf_mode=DoubleRow: 2 k-tiles per matmul at ~2x rate.
Genes are assigned per-core by masked-weight row energy: the 468 genes
with the LEAST energy go to fp8, so the fp8 quantization noise lands on
the smallest share of the output variance (~0.014 measured, vs 0.033 for
all-fp8). fp8 operands are pre-scaled (x by 2^4, weights by 2^14) to sit
in e4m3's normal range; the bf16 operands carry the same power-of-2
scales so every matmul accumulates into one PSUM group at scale 2^18,
which the host's final sum divides back out.

Startup: per-k-tile weight DMAs + m-tiles processed in pairs with the
k-loop interleaved across two PSUM banks, so the PE starts ~3us in and
consumes weight tiles as they land. A short N=128 garbage-matmul burst
warms the PE HAM clock gate during the initial DMA window. Outputs are
bf16 partials (halves output DMA bytes; host sums in fp32).

Per-core layout (host-packed for DMA friendliness + TensorE layout):
  xt:  (32, 128, 1408) bf16   xt[m, p, k*128+c] = xpad[m*128+c, k*128+p]
       -> SBUF tile [128, 1408]; slice [:, k*128:(k+1)*128] is the
          stationary lhsT (K=128 genes, M=128 batch) for (m, k)
  xf8: (32, 128, 512) e4m3    same pattern over the 4 fp8 k-tiles
       -> SBUF tile [128, 4, 128]; [:, 2j:2j+2, :] is the DoubleRow lhsT
  mw:  (11, 128, 500) bf16    mw[k, p, n] = mwpad[k*128+p, n]
  mf8: (128, 4, 512) e4m3     pathways padded 500->512 so the DoubleRow
       pair stride (512B) meets the 16B-multiple constraint
  out: (32, 128, 500) bf16 partial (scaled by 2^18), PSUM-accumulated.
"""

import functools
import os

import ml_dtypes
import numpy as np

B, G, P = 4096, 15000, 500
N_CORES = 8
GS = G // N_CORES          # 1875 genes per core
KT = 128                   # k-tile size (partition dim)
MT = 128                   # batch tile
NM = B // MT               # 32 batch tiles
NKB = 11                   # bf16 k-tiles per core
NKF = 4                    # fp8 k-tiles per core (must be even)
KPB = NKB * KT             # 1408 bf16 gene slots (1407 genes + bias)
KPF = NKF * KT             # 512 fp8 gene slots (468 genes + 44 zero)
NGB = KPB - 1              # real genes in the bf16 region
NGF = GS - NGB             # real genes in the fp8 region
PF = 512                   # fp8 pathway stride (500 padded to 512)
S_X = 16.0                 # 2^4  x scale (both precisions)
S_W = 16384.0              # 2^14 weight scale (both precisions)
UNSCALE = 1.0 / (S_X * S_W)
NJUNK = 34                 # HAM warm-up matmuls (N=128, ~107ns each cold)

_BF16 = ml_dtypes.bfloat16
_FP8 = ml_dtypes.float8_e4m3

LAST_EXEC_TIME_NS = None
LAST_TRACE = None
LAST_RESULTS = None


def _install_profshim():
    """Make run_bass_kernel_spmd(trace=True) work in the axon container:
    recreate the antenv.axon_hooks NTFF hook + keep artifacts local."""
    import sys
    import types

    if "antenv.axon_hooks" not in sys.modules:
        import antenv
        from trn_agent_boot.trn_boot import _ntff_profile_via_ctypes

        mod = types.ModuleType("antenv.axon_hooks")
        mod._hook = _ntff_profile_via_ctypes("/opt/axon/libaxon_pjrt.so")
        mod.set_axon_ntff_profile_hook = lambda h: setattr(mod, "_hook", h)
        mod.get_axon_ntff_profile_hook = lambda: mod._hook
        sys.modules["antenv.axon_hooks"] = mod
        antenv.axon_hooks = mod

    import concourse.bass_utils as bu

    bu.upload_artifacts = lambda tmpdir: f"file://{tmpdir}"


@functools.lru_cache(maxsize=1)
def _build():
    import concourse.bass as bass
    import concourse.mybir as mybir
    import concourse.tile as tile
    from concourse import bacc

    nc = bacc.Bacc(
        "TRN2", target_bir_lowering=False, debug=False, num_devices=N_CORES
    )
    bf16 = mybir.dt.bfloat16
    fp8 = mybir.dt.float8e4
    f32 = mybir.dt.float32
    DR = mybir.MatmulPerfMode.DoubleRow
    xt_d = nc.dram_tensor("xt", [NM, KT, KPB], bf16, kind="ExternalInput")
    xf8_d = nc.dram_tensor("xf8", [NM, KT, KPF], fp8, kind="ExternalInput")
    mw_d = nc.dram_tensor("mw", [NKB, KT, P], bf16, kind="ExternalInput")
    mf8_d = nc.dram_tensor("mf8", [KT, NKF * PF], fp8, kind="ExternalInput")
    out_d = nc.dram_tensor("out", [NM, MT, P], bf16, kind="ExternalOutput")

    HALF = 6 * KT  # split xt tiles at the k=6 boundary
    QUAD = 4       # m-tiles per group (one PSUM bank each, all 8 banks used)
    NQ = NM // QUAD

    with tile.TileContext(nc) as tc:
        with (
            tc.tile_pool(name="wpool", bufs=1) as wpool,
            tc.tile_pool(name="xpool", bufs=8) as xpool,
            tc.tile_pool(name="fpool", bufs=8) as fpool,
            tc.tile_pool(name="opool", bufs=8) as opool,
            tc.tile_pool(name="pspool", bufs=2, space=bass.MemorySpace.PSUM) as pspool,
        ):
            # Warm the PE HAM clock gate during the initial DMA window:
            # the first weight tile cannot land before ~11us (engine
            # preamble + first-DMA latency), so N=128 garbage matmuls keep
            # the PE's 4096-cycle activity window continuously busy until
            # then -- the real stream starts at 2.4GHz instead of 1.2.
            # The junk target borrows the psA ring slot (start=True on the
            # first real matmul resets it).
            junk = wpool.tile([KT, KT], bf16)
            nc.gpsimd.memset(junk[:], 0.0)
            jps = pspool.tile([MT, P], f32, tag="psA")
            for _ in range(NJUNK):
                nc.tensor.matmul(jps[:, :KT], junk[:], junk[:], start=True, stop=True)

            # Masked weights land per-k-tile (128KB each) so matmul k can
            # start as soon as tile k arrives. mw[0] rides the sync ring
            # (its first transfer) while mw[1..] stream on the scalar ring
            # in parallel, minimizing time-to-first-matmul; the fp8 tiles
            # are consumed last so their DMA goes last.
            mw = wpool.tile([KT, NKB * P], bf16)
            mf8 = wpool.tile([KT, NKF, PF], fp8)
            nc.scalar.dma_start(mf8[:], mf8_d[:])
            nc.sync.dma_start(mw[:, 0:P], mw_d[0])
            for k in range(1, NKB):
                nc.scalar.dma_start(mw[:, k * P : (k + 1) * P], mw_d[k])

            # m-tiles in quads with the k-loop interleaved across four
            # PSUM banks: during startup each arriving weight tile feeds
            # four matmuls (~845ns of PE work per ~420ns of DMA), and the
            # fp8 DoubleRow block's bf16<->fp8 mode-switch drain (~284ns)
            # is paid once per quad instead of once per pair.
            for q in range(NQ):
                ms = [QUAD * q + i for i in range(QUAD)]
                xts, f8s, pss, ots = [], [], [], []
                for i in range(QUAD):
                    xts.append(xpool.tile([KT, KPB], bf16, name=f"xt{i}"))
                    f8s.append(fpool.tile([KT, NKF, MT], fp8, name=f"f8{i}"))
                    pss.append(
                        pspool.tile([MT, P], f32, tag=f"ps{'ABCD'[i]}", name=f"ps{i}")
                    )
                    ots.append(opool.tile([MT, P], bf16, name=f"ot{i}"))
                # Even quads run their fp8 DoubleRow block first, odd
                # quads last, so consecutive quads' DR blocks abut and the
                # ~284ns fp8->bf16 PE mode-switch drain is paid once per
                # TWO quads. For quad 0, DR-first has a second benefit:
                # mf8 + the four f8 tiles are only 0.5MB, so the PE gets
                # real work ~2us before enough bf16 x-tiles streamed in.
                dr_first = q % 2 == 0
                if q == 0:
                    for i in range(QUAD):
                        nc.sync.dma_start(f8s[i][:], xf8_d[ms[i]])
                    # fine-grained halves so the k-loop can start on h0
                    for i in range(QUAD):
                        nc.sync.dma_start(xts[i][:, :HALF], xt_d[ms[i]][:, :HALF])
                    for i in range(QUAD):
                        nc.sync.dma_start(xts[i][:, HALF:], xt_d[ms[i]][:, HALF:])
                else:
                    for i in range(QUAD):
                        nc.sync.dma_start(xts[i][:], xt_d[ms[i]])
                    for i in range(QUAD):
                        nc.sync.dma_start(f8s[i][:], xf8_d[ms[i]])

                def dr_block(start):
                    # grouped per m-tile so when it runs last, each tile's
                    # stop lands as early as possible and its PSUM copy
                    # overlaps the remaining DoubleRow matmuls
                    for i in range(QUAD):
                        for j in range(NKF // 2):
                            nc.tensor.matmul(
                                pss[i][:],
                                f8s[i][:, 2 * j : 2 * j + 2, :],
                                mf8[:, 2 * j : 2 * j + 2, 0:P],
                                start=start and j == 0,
                                stop=(not start) and j == NKF // 2 - 1,
                                perf_mode=DR,
                            )

                if dr_first:
                    dr_block(start=True)
                for k in range(NKB):
                    for i in range(QUAD):
                        nc.tensor.matmul(
                            pss[i][:],
                            xts[i][:, k * MT : (k + 1) * MT],
                            mw[:, k * P : (k + 1) * P],
                            start=(not dr_first) and k == 0,
                            stop=dr_first and k == NKB - 1,
                        )
                if not dr_first:
                    dr_block(start=False)
                # All copies are emitted before any out-DMA issue: the
                # ~600ns HWDGE trigger ops otherwise sit between copies on
                # the scalar queue and head-of-line-block copyD, stalling
                # the next quad's first matmuls (which wait on PSUM-ring
                # copy semaphores).
                last_q = q == NQ - 1
                for i in range(QUAD):
                    if last_q and i == QUAD - 1:
                        # split the final copy across both engines so the
                        # tail's copy latency halves
                        nc.vector.tensor_copy(ots[i][:, : P // 2], pss[i][:, : P // 2])
                        nc.scalar.copy(ots[i][:, P // 2 :], pss[i][:, P // 2 :])
                    elif i % 2 == 0:
                        nc.vector.tensor_copy(ots[i][:], pss[i][:])
                    else:
                        nc.scalar.copy(ots[i][:], pss[i][:])
                for i in range(QUAD):
                    # last quad: spread issues across both HWDGE rings so
                    # the final DMA triggers + completion receipts overlap
                    if last_q and i >= QUAD - 2:
                        nc.sync.dma_start(out_d[ms[i]], ots[i][:])
                    else:
                        nc.scalar.dma_start(out_d[ms[i]], ots[i][:])
    nc.compile()
    return nc


def _pack_inputs(x, weight, mask, bias):
    """Host-side shard, precision-split and pre-tile per core."""
    xf = np.asarray(x, dtype=np.float32)
    wf = np.asarray(weight, dtype=np.float32)
    mf = np.asarray(mask, dtype=np.float32)
    bf = np.asarray(bias, dtype=np.float32)
    mwT = wf.T * mf  # (G, P) premultiplied masked weights

    in_maps = []
    for core in range(N_CORES):
        g0 = core * GS
        mwc = mwT[g0 : g0 + GS]              # (GS, P)
        energy = np.einsum("gp,gp->g", mwc, mwc)
        order = np.argsort(energy)
        light = order[:NGF]                  # lowest-energy genes -> fp8
        heavy = order[NGF:]                  # the rest -> bf16

        # bf16 side: 1407 genes + bias column, scaled by S_X / S_W
        xpad = np.zeros((B, KPB), dtype=_BF16)
        xpad[:, :NGB] = (xf[:, g0 + heavy] * S_X).astype(_BF16)
        xpad[:, NGB] = _BF16(S_X)            # bias column
        xt = np.ascontiguousarray(
            xpad.reshape(NM, MT, NKB, KT).transpose(0, 3, 2, 1)
        ).reshape(NM, KT, NKB * MT)

        mwpad = np.zeros((KPB, P), dtype=np.float32)
        mwpad[:NGB] = mwc[heavy] * S_W
        if core == 0:
            mwpad[NGB] = bf * S_W            # bias row (once across cores)
        mw = mwpad.reshape(NKB, KT, P).astype(_BF16)

        # fp8 side: 468 lightest genes + zero pads, e4m3 with the same scales
        x8pad = np.zeros((B, KPF), dtype=_FP8)
        x8pad[:, :NGF] = np.clip(xf[:, g0 + light] * S_X, -240, 240).astype(_FP8)
        xf8 = np.ascontiguousarray(
            x8pad.reshape(NM, MT, NKF, KT).transpose(0, 3, 2, 1)
        ).reshape(NM, KT, NKF * MT)

        m8pad = np.zeros((KPF, PF), dtype=np.float32)
        m8pad[:NGF, :P] = mwc[light] * S_W
        mf8 = np.ascontiguousarray(
            np.clip(m8pad, -240, 240)
            .astype(_FP8)
            .reshape(NKF, KT, PF)
            .transpose(1, 0, 2)
        ).reshape(KT, NKF * PF)

        in_maps.append({"xt": xt, "xf8": xf8, "mw": mw, "mf8": mf8})
    return in_maps


def kernel(x, weight, mask, bias):
    global LAST_EXEC_TIME_NS, LAST_TRACE, LAST_RESULTS

    profile = bool(int(os.environ.get("KERNEL_PROFILE", "0")))
    if profile:
        _install_profshim()

    nc = _build()
    in_maps = _pack_inputs(x, weight, mask, bias)

    from concourse.bass_utils import run_bass_kernel_spmd

    tmpdir = None
    if profile:
        import tempfile

        base = os.environ.get("KERNEL_TRACE_DIR")
        if base:
            os.makedirs(base, exist_ok=True)
        tmpdir = tempfile.mkdtemp(prefix="ktrace_", dir=base)

    res = run_bass_kernel_spmd(
        nc,
        in_maps,
        core_ids=list(range(N_CORES)),
        trace=profile,
        tmpdir=tmpdir,
    )
    LAST_EXEC_TIME_NS = res.exec_time_ns
    LAST_TRACE = (
        res.instructions_and_trace[1] if res.instructions_and_trace else None
    )
    LAST_RESULTS = res

    parts = np.stack(
        [r["out"].astype(np.float32).reshape(B, P) for r in res.results]
    )
    return parts.sum(axis=0, dtype=np.float32) * np.float32(UNSCALE)


# revision 20
# speedup vs baseline: 1.0361x; 1.0361x over previous
"""Trainium2 Bass kernel for nn_NetworkActivity_layer (masked linear):

    out = x @ (weight * mask.T).T + bias      x:(4096,15000) w:(500,15000)
                                              mask:(15000,500) bias:(500,)

Strategy: shard the contraction (gene) dim K=15000 across 8 NeuronCores
(1875 genes/core). Each core computes a partial (4096,500) output; the
host sums the 8 partials (the K-shard "unshard" step). The bias rides in
an extra padded gene (x column of ones, masked-weight row = bias on core
0), so the device kernel computes the complete affine map.

The masked weights are premultiplied on the host (weight * mask.T) so the
device does nothing but matmuls. The kernel is PE-streaming-bound, so the
contraction is split by precision to cut PE cycles while holding the
error budget (rel err gate 2e-2):
  - 11 k-tiles (1407 genes + bias) in bf16: 1 matmul each, N=500.
  - 4 k-tiles (468 genes + 44 zero pads) in fp8 e4m3 with
    perf_mode=DoubleRow: 2 k-tiles per matmul at ~2x rate.
Genes are assigned per-core by masked-weight row energy: the 468 genes
with the LEAST energy go to fp8, so the fp8 quantization noise lands on
the smallest share of the output variance (~0.014 measured, vs 0.033 for
all-fp8). fp8 operands are pre-scaled (x by 2^4, weights by 2^14) to sit
in e4m3's normal range; the bf16 operands carry the same power-of-2
scales so every matmul accumulates into one PSUM group at scale 2^18,
which the host's final sum divides back out.

Startup: per-k-tile weight DMAs + m-tiles processed in pairs with the
k-loop interleaved across two PSUM banks, so the PE starts ~3us in and
consumes weight tiles as they land. A short N=128 garbage-matmul burst
warms the PE HAM clock gate during the initial DMA window. Outputs are
bf16 partials (halves output DMA bytes; host sums in fp32).

Per-core layout (host-packed for DMA friendliness + TensorE layout):
  xt:  (32, 128, 1408) bf16   xt[m, p, k*128+c] = xpad[m*128+c, k*128+p]
       -> SBUF tile [128, 1408]; slice [:, k*128:(k+1)*128] is the
          stationary lhsT (K=128 genes, M=128 batch) for (m, k)
  xf8: (32, 128, 512) e4m3    same pattern over the 4 fp8 k-tiles
       -> SBUF tile [128, 4, 128]; [:, 2j:2j+2, :] is the DoubleRow lhsT
  mw:  (11, 128, 500) bf16    mw[k, p, n] = mwpad[k*128+p, n]
  mf8: (128, 4, 512) e4m3     pathways padded 500->512 so the DoubleRow
       pair stride (512B) meets the 16B-multiple constraint
  out: (32, 128, 500) bf16 partial (scaled by 2^18), PSUM-accumulated.
"""

import functools
import os

import ml_dtypes
import numpy as np

B, G, P = 4096, 15000, 500
N_CORES = 8
GS = G // N_CORES          # 1875 genes per core
KT = 128                   # k-tile size (partition dim)
MT = 128                   # batch tile
NM = B // MT               # 32 batch tiles
NKB = 11                   # bf16 k-tiles per core
NKF = 4                    # fp8 k-tiles per core (must be even)
KPB = NKB * KT             # 1408 bf16 gene slots (1407 genes + bias)
KPF = NKF * KT             # 512 fp8 gene slots (468 genes + 44 zero)
NGB = KPB - 1              # real genes in the bf16 region
NGF = GS - NGB             # real genes in the fp8 region
PF = 512                   # fp8 pathway stride (500 padded to 512)
S_X = 16.0                 # 2^4  x scale (both precisions)
S_W = 16384.0              # 2^14 weight scale (both precisions)
UNSCALE = 1.0 / (S_X * S_W)
NJUNK = 34                 # HAM warm-up matmuls (N=128, ~107ns each cold)

_BF16 = ml_dtypes.bfloat16
_FP8 = ml_dtypes.float8_e4m3

LAST_EXEC_TIME_NS = None
LAST_TRACE = None
LAST_RESULTS = None


def _install_profshim():
    """Make run_bass_kernel_spmd(trace=True) work in the axon container:
    recreate the antenv.axon_hooks NTFF hook + keep artifacts local."""
    import sys
    import types

    if "antenv.axon_hooks" not in sys.modules:
        import antenv
        from trn_agent_boot.trn_boot import _ntff_profile_via_ctypes

        mod = types.ModuleType("antenv.axon_hooks")
        mod._hook = _ntff_profile_via_ctypes("/opt/axon/libaxon_pjrt.so")
        mod.set_axon_ntff_profile_hook = lambda h: setattr(mod, "_hook", h)
        mod.get_axon_ntff_profile_hook = lambda: mod._hook
        sys.modules["antenv.axon_hooks"] = mod
        antenv.axon_hooks = mod

    import concourse.bass_utils as bu

    bu.upload_artifacts = lambda tmpdir: f"file://{tmpdir}"


@functools.lru_cache(maxsize=1)
def _build():
    import concourse.bass as bass
    import concourse.mybir as mybir
    import concourse.tile as tile
    from concourse import bacc

    nc = bacc.Bacc(
        "TRN2", target_bir_lowering=False, debug=False, num_devices=N_CORES
    )
    bf16 = mybir.dt.bfloat16
    fp8 = mybir.dt.float8e4
    f32 = mybir.dt.float32
    DR = mybir.MatmulPerfMode.DoubleRow
    xt_d = nc.dram_tensor("xt", [NM, KT, KPB], bf16, kind="ExternalInput")
    xf8_d = nc.dram_tensor("xf8", [NM, KT, KPF], fp8, kind="ExternalInput")
    mw_d = nc.dram_tensor("mw", [NKB, KT, P], bf16, kind="ExternalInput")
    mf8_d = nc.dram_tensor("mf8", [KT, NKF * PF], fp8, kind="ExternalInput")
    out_d = nc.dram_tensor("out", [NM, MT, P], bf16, kind="ExternalOutput")

    HALF = 6 * KT  # split xt tiles at the k=6 boundary
    QUAD = 4       # m-tiles per group (one PSUM bank each, all 8 banks used)
    NQ = NM // QUAD

    with tile.TileContext(nc) as tc:
        with (
            tc.tile_pool(name="wpool", bufs=1) as wpool,
            tc.tile_pool(name="xpool", bufs=8) as xpool,
            tc.tile_pool(name="fpool", bufs=8) as fpool,
            tc.tile_pool(name="opool", bufs=8) as opool,
            tc.tile_pool(name="pspool", bufs=2, space=bass.MemorySpace.PSUM) as pspool,
        ):
            # Warm the PE HAM clock gate during the initial DMA window:
            # the first weight tile cannot land before ~11us (engine
            # preamble + first-DMA latency), so N=128 garbage matmuls keep
            # the PE's 4096-cycle activity window continuously busy until
            # then -- the real stream starts at 2.4GHz instead of 1.2.
            # The junk target borrows the psA ring slot (start=True on the
            # first real matmul resets it).
            junk = wpool.tile([KT, KT], bf16)
            nc.gpsimd.memset(junk[:], 0.0)
            jps = pspool.tile([MT, P], f32, tag="psA")
            for _ in range(NJUNK):
                nc.tensor.matmul(jps[:, :KT], junk[:], junk[:], start=True, stop=True)

            # Masked weights land per-k-tile (128KB each) so matmul k can
            # start as soon as tile k arrives. mw[0] rides the sync ring
            # (its first transfer) while mw[1..] stream on the scalar ring
            # in parallel, minimizing time-to-first-matmul; the fp8 tiles
            # are consumed last so their DMA goes last.
            mw = wpool.tile([KT, NKB * P], bf16)
            mf8 = wpool.tile([KT, NKF, PF], fp8)
            nc.scalar.dma_start(mf8[:], mf8_d[:])
            nc.sync.dma_start(mw[:, 0:P], mw_d[0])
            for k in range(1, NKB):
                nc.scalar.dma_start(mw[:, k * P : (k + 1) * P], mw_d[k])

            # m-tiles in quads with the k-loop interleaved across four
            # PSUM banks: during startup each arriving weight tile feeds
            # four matmuls (~845ns of PE work per ~420ns of DMA), and the
            # fp8 DoubleRow block's bf16<->fp8 mode-switch drain (~284ns)
            # is paid once per quad instead of once per pair.
            for q in range(NQ):
                ms = [QUAD * q + i for i in range(QUAD)]
                xts, f8s, pss, ots = [], [], [], []
                for i in range(QUAD):
                    xts.append(xpool.tile([KT, KPB], bf16, name=f"xt{i}"))
                    f8s.append(fpool.tile([KT, NKF, MT], fp8, name=f"f8{i}"))
                    pss.append(
                        pspool.tile([MT, P], f32, tag=f"ps{'ABCD'[i]}", name=f"ps{i}")
                    )
                    ots.append(opool.tile([MT, P], bf16, name=f"ot{i}"))
                # Even quads run their fp8 DoubleRow block first, odd
                # quads last, so consecutive quads' DR blocks abut and the
                # ~284ns fp8->bf16 PE mode-switch drain is paid once per
                # TWO quads. For quad 0, DR-first has a second benefit:
                # mf8 + the four f8 tiles are only 0.5MB, so the PE gets
                # real work ~2us before enough bf16 x-tiles streamed in.
                dr_first = q % 2 == 0
                if q == 0:
                    for i in range(QUAD):
                        nc.sync.dma_start(f8s[i][:], xf8_d[ms[i]])
                    # fine-grained halves so the k-loop can start on h0
                    for i in range(QUAD):
                        nc.sync.dma_start(xts[i][:, :HALF], xt_d[ms[i]][:, :HALF])
                    for i in range(QUAD):
                        nc.sync.dma_start(xts[i][:, HALF:], xt_d[ms[i]][:, HALF:])
                elif dr_first:
                    for i in range(QUAD):
                        nc.sync.dma_start(f8s[i][:], xf8_d[ms[i]])
                    for i in range(QUAD):
                        nc.sync.dma_start(xts[i][:], xt_d[ms[i]])
                else:
                    for i in range(QUAD):
                        nc.sync.dma_start(xts[i][:], xt_d[ms[i]])
                    for i in range(QUAD):
                        nc.sync.dma_start(f8s[i][:], xf8_d[ms[i]])

                def dr_block(start):
                    # grouped per m-tile so when it runs last, each tile's
                    # stop lands as early as possible and its PSUM copy
                    # overlaps the remaining DoubleRow matmuls
                    for i in range(QUAD):
                        for j in range(NKF // 2):
                            nc.tensor.matmul(
                                pss[i][:],
                                f8s[i][:, 2 * j : 2 * j + 2, :],
                                mf8[:, 2 * j : 2 * j + 2, 0:P],
                                start=start and j == 0,
                                stop=(not start) and j == NKF // 2 - 1,
                                perf_mode=DR,
                            )

                if dr_first:
                    dr_block(start=True)
                for k in range(NKB):
                    for i in range(QUAD):
                        nc.tensor.matmul(
                            pss[i][:],
                            xts[i][:, k * MT : (k + 1) * MT],
                            mw[:, k * P : (k + 1) * P],
                            start=(not dr_first) and k == 0,
                            stop=dr_first and k == NKB - 1,
                        )
                if not dr_first:
                    dr_block(start=False)
                # All copies are emitted before any out-DMA issue: the
                # ~600ns HWDGE trigger ops otherwise sit between copies on
                # the scalar queue and head-of-line-block copyD, stalling
                # the next quad's first matmuls (which wait on PSUM-ring
                # copy semaphores).
                last_q = q == NQ - 1
                for i in range(QUAD):
                    if last_q and i == QUAD - 1:
                        # split the final copy across both engines so the
                        # tail's copy latency halves
                        nc.vector.tensor_copy(ots[i][:, : P // 2], pss[i][:, : P // 2])
                        nc.scalar.copy(ots[i][:, P // 2 :], pss[i][:, P // 2 :])
                    elif i % 2 == 0:
                        nc.vector.tensor_copy(ots[i][:], pss[i][:])
                    else:
                        nc.scalar.copy(ots[i][:], pss[i][:])
                for i in range(QUAD):
                    # last quad: spread issues across both HWDGE rings so
                    # the final DMA triggers + completion receipts overlap
                    if last_q and i >= QUAD - 2:
                        nc.sync.dma_start(out_d[ms[i]], ots[i][:])
                    else:
                        nc.scalar.dma_start(out_d[ms[i]], ots[i][:])
    nc.compile()
    return nc


def _pack_inputs(x, weight, mask, bias):
    """Host-side shard, precision-split and pre-tile per core."""
    xf = np.asarray(x, dtype=np.float32)
    wf = np.asarray(weight, dtype=np.float32)
    mf = np.asarray(mask, dtype=np.float32)
    bf = np.asarray(bias, dtype=np.float32)
    mwT = wf.T * mf  # (G, P) premultiplied masked weights

    in_maps = []
    for core in range(N_CORES):
        g0 = core * GS
        mwc = mwT[g0 : g0 + GS]              # (GS, P)
        energy = np.einsum("gp,gp->g", mwc, mwc)
        order = np.argsort(energy)
        light = order[:NGF]                  # lowest-energy genes -> fp8
        heavy = order[NGF:]                  # the rest -> bf16

        # bf16 side: 1407 genes + bias column, scaled by S_X / S_W
        xpad = np.zeros((B, KPB), dtype=_BF16)
        xpad[:, :NGB] = (xf[:, g0 + heavy] * S_X).astype(_BF16)
        xpad[:, NGB] = _BF16(S_X)            # bias column
        xt = np.ascontiguousarray(
            xpad.reshape(NM, MT, NKB, KT).transpose(0, 3, 2, 1)
        ).reshape(NM, KT, NKB * MT)

        mwpad = np.zeros((KPB, P), dtype=np.float32)
        mwpad[:NGB] = mwc[heavy] * S_W
        if core == 0:
            mwpad[NGB] = bf * S_W            # bias row (once across cores)
        mw = mwpad.reshape(NKB, KT, P).astype(_BF16)

        # fp8 side: 468 lightest genes + zero pads, e4m3 with the same scales
        x8pad = np.zeros((B, KPF), dtype=_FP8)
        x8pad[:, :NGF] = np.clip(xf[:, g0 + light] * S_X, -240, 240).astype(_FP8)
        xf8 = np.ascontiguousarray(
            x8pad.reshape(NM, MT, NKF, KT).transpose(0, 3, 2, 1)
        ).reshape(NM, KT, NKF * MT)

        m8pad = np.zeros((KPF, PF), dtype=np.float32)
        m8pad[:NGF, :P] = mwc[light] * S_W
        mf8 = np.ascontiguousarray(
            np.clip(m8pad, -240, 240)
            .astype(_FP8)
            .reshape(NKF, KT, PF)
            .transpose(1, 0, 2)
        ).reshape(KT, NKF * PF)

        in_maps.append({"xt": xt, "xf8": xf8, "mw": mw, "mf8": mf8})
    return in_maps


def kernel(x, weight, mask, bias):
    global LAST_EXEC_TIME_NS, LAST_TRACE, LAST_RESULTS

    profile = bool(int(os.environ.get("KERNEL_PROFILE", "0")))
    if profile:
        _install_profshim()

    nc = _build()
    in_maps = _pack_inputs(x, weight, mask, bias)

    from concourse.bass_utils import run_bass_kernel_spmd

    tmpdir = None
    if profile:
        import tempfile

        base = os.environ.get("KERNEL_TRACE_DIR")
        if base:
            os.makedirs(base, exist_ok=True)
        tmpdir = tempfile.mkdtemp(prefix="ktrace_", dir=base)

    res = run_bass_kernel_spmd(
        nc,
        in_maps,
        core_ids=list(range(N_CORES)),
        trace=profile,
        tmpdir=tmpdir,
    )
    LAST_EXEC_TIME_NS = res.exec_time_ns
    LAST_TRACE = (
        res.instructions_and_trace[1] if res.instructions_and_trace else None
    )
    LAST_RESULTS = res

    parts = np.stack(
        [r["out"].astype(np.float32).reshape(B, P) for r in res.results]
    )
    return parts.sum(axis=0, dtype=np.float32) * np.float32(UNSCALE)
